# revision 35
# baseline (speedup 1.0000x reference)
"""DiversityDensity kernel for 8x Trainium2 NeuronCores.

Math: for each row u of U_z:
    dens(u)  = -0.5*||u||^2 - 0.5*NZ*log(2*pi)
    div(u)   = min_l ||u - l||_2  over rows l of L_z
    dd       = exp(dens + log(div + eps)); dd = (dd - min dd) / (max dd + eps)

Device computes m(u) = min_l (||l||^2 - 2 u.l) via a K=34 fp16 matmul
(32 features + split ||l||^2 hi/lo rows against a ones pair) streamed
over L in 512-row chunks, with a running elementwise min over the PSUM
tiles.  Layout: L-rows on output partitions (M), u on the free dim (N),
so the final reduce is a cheap cross-partition min done on host after
gather.

The PE in this environment is pinned at the cold 1.2 GHz HAM clock, so
K=34 matmuls use 64x64 array tiling: four M=64 matmuls (two groups x
two PSUM partition halves) co-stream in the four array quadrants,
recovering 4x column throughput.  Each PSUM tile interleaves TWO
groups' chunks in its column blocks so a full 4-tile quad fills ONE
psum tile and the 2-slot fill/drain pipeline survives.

PSUM drain is the throughput bound; it is split across two routes:
  A: DVE tensor_tensor(min) fp32 straight from PSUM into acc32
  B: ScalarE copy PSUM->SBUF fp16, then DVE min at 2x into acc16
with N_A tuned so DVE and ScalarE finish together.
d(u)^2 = ||u||^2 + m(u); sqrt/exp/normalize on host (O(N_U) work).

Sharding: U_z rows split 8 ways (512 rows/core); L_z (8 MB) replicated.
Measured: ~246 us HW exec, L2 rel err ~6e-5 (fp16 operand rounding).
"""

import numpy as np

N_U, N_L, NZ = 4096, 65536, 32
CORES = 8
SHARD = N_U // CORES  # 512
K = NZ + 2  # 34: 32 features + c_hi + c_lo rows
NCHUNK = 512  # L-rows per group (4 matmuls of M=128)
GROUPS = N_L // NCHUNK  # 128
LOG_2PI = float(np.log(2.0 * np.pi))
EPS = 1e-18

# Per-group drain route: A = DVE tensor_tensor(min) fp32 from PSUM,
# B = ScalarE copy to SBUF fp16 + DVE min at 2x.  Balances DVE vs ACT.
N_A = 33

TRACE = False
LAST = {}

_CACHE = {}


def _route_a(g: int) -> bool:
    # Last chunks go to route A so the fp16 tail folds overlap the
    # final fp32 drains instead of serializing after them.
    if g >= GROUPS - 4:
        return True
    n = N_A - 4
    m = GROUPS - 4
    return (g + 1) * n // m > g * n // m


def _build():
    import concourse.bass as bass  # noqa: F401
    import concourse.tile as tile
    from concourse import bacc, mybir

    f32 = mybir.dt.float32
    f16 = mybir.dt.float16
    MIN = mybir.AluOpType.min

    nc = bacc.Bacc(
        "TRN2", target_bir_lowering=False, debug=False, num_devices=CORES
    )
    ut_d = nc.declare_dram_parameter("ut", [K, SHARD], f16, isOutput=False)
    lt_d = nc.declare_dram_parameter("lt", [GROUPS, K, NCHUNK], f16, isOutput=False)
    out_d = nc.declare_dram_parameter("partmin", [128, SHARD], f32, isOutput=True)

    with tile.TileContext(nc) as tc:
        with (
            tc.tile_pool(name="const", bufs=1) as cpool,
            tc.tile_pool(name="ltp", bufs=10) as ltpool,
            tc.tile_pool(name="accp", bufs=1) as accpool,
            tc.tile_pool(name="s16p", bufs=10) as s16pool,
            tc.tile_pool(name="tailp", bufs=1) as tailpool,
            tc.tile_pool(name="psum", bufs=2, space="PSUM") as pspool,
        ):
            # rhs (U^T in fp16, plus two ones rows) at both PE row-group bases
            ut_t = cpool.tile([64 + K, SHARD], f16)
            nc.sync.dma_start(ut_t[0:K, :], ut_d[:, :])
            nc.sync.dma_start(ut_t[64 : 64 + K, :], ut_d[:, :])

            # Two fp16 accumulators alternate across route-B chunks so
            # consecutive DVE mins don't RAW-chain (hides the DVE pipe
            # drain); memsets go to GpSimd to keep DVE free.
            acc32 = accpool.tile([128, 2048], f32)
            acc16a = accpool.tile([128, 2048], f16)
            acc16b = accpool.tile([128, 2048], f16)
            nc.gpsimd.memset(acc32[:], 3.0e38)
            nc.gpsimd.memset(acc16a[:], 60000.0)
            nc.gpsimd.memset(acc16b[:], 60000.0)
            accs16 = (acc16a, acc16b)
            nb = 0

            # 64x64 array tiling: each 128-l-row chunk becomes two M=64
            # matmuls into disjoint PSUM partition halves; even/odd groups
            # sit in SBUF partition halves 0/64.  Emitting group PAIRS with
            # chunk-interleaved matmuls puts all four array tiles
            # (0,0),(0,64),(64,0),(64,64) in flight -> 4x co-streaming,
            # recovering full throughput at the cold PE clock.
            for gp in range(GROUPS // 2):
                g0, g1 = 2 * gp, 2 * gp + 1
                lt0 = ltpool.tile([64 + K, NCHUNK], f16, tag="lt")
                nc.sync.dma_start(lt0[0:K, :], lt_d[g0])
                lt1 = ltpool.tile([64 + K, NCHUNK], f16, tag="lt")
                nc.sync.dma_start(lt1[64 : 64 + K, :], lt_d[g1])

                # Each psum tile holds TWO chunks of EACH group (column
                # blocks: [g0.c, g0.c+1, g1.c, g1.c+1]) so a full 4-tile
                # quad streams into ONE psum tile and the 2-slot pipeline
                # (PE fills one tile while the other drains) is preserved.
                for half in range(2):
                    ps = pspool.tile([128, 2048], f32, tag="ps")
                    for cc in range(2):
                        c = 2 * half + cc
                        for bi, (base, lt_t) in enumerate(((0, lt0), (64, lt1))):
                            for h in range(2):
                                nc.tensor.matmul(
                                    ps[
                                        64 * h : 64 * h + 64,
                                        (2 * bi + cc) * 512 : (2 * bi + cc + 1) * 512,
                                    ],
                                    lhsT=lt_t[
                                        base : base + K,
                                        c * 128 + 64 * h : c * 128 + 64 * h + 64,
                                    ],
                                    rhs=ut_t[base : base + K, :],
                                    start=True,
                                    stop=True,
                                )
                    if _route_a(2 * gp + half):
                        nc.vector.tensor_tensor(acc32[:], acc32[:], ps[:], MIN)
                    else:
                        s16 = s16pool.tile([128, 2048], f16, tag="s16")
                        nc.scalar.copy(s16[:], ps[:])
                        acc = accs16[nb]
                        nb ^= 1
                        nc.vector.tensor_tensor(acc[:], acc[:], s16[:], MIN)

            # Tail: fold the 4 sub-chunk columns together, merge routes.
            acc16 = tailpool.tile([128, 2048], f16)
            nc.vector.tensor_tensor(acc16[:], acc16a[:], acc16b[:], MIN)
            t16a = tailpool.tile([128, 1024], f16)
            nc.vector.tensor_tensor(t16a[:], acc16[:, 0:1024], acc16[:, 1024:2048], MIN)
            t16b = tailpool.tile([128, 512], f16)
            nc.vector.tensor_tensor(t16b[:], t16a[:, 0:512], t16a[:, 512:1024], MIN)
            t16c = tailpool.tile([128, 512], f32)
            nc.vector.tensor_copy(t16c[:], t16b[:])

            t32a = tailpool.tile([128, 1024], f32)
            nc.vector.tensor_tensor(t32a[:], acc32[:, 0:1024], acc32[:, 1024:2048], MIN)
            t32b = tailpool.tile([128, 512], f32)
            nc.vector.tensor_tensor(t32b[:], t32a[:, 0:512], t32a[:, 512:1024], MIN)

            res = tailpool.tile([128, 512], f32)
            nc.vector.tensor_tensor(res[:], t32b[:], t16c[:], MIN)
            nc.sync.dma_start(out_d[:, :], res[:])

    nc.compile()
    return nc


def _get_nc():
    if "nc" not in _CACHE:
        _CACHE["nc"] = _build()
    return _CACHE["nc"]


def kernel(pred: np.ndarray, U_z: np.ndarray, L_z: np.ndarray) -> np.ndarray:
    from concourse.bass_utils import run_bass_kernel_spmd

    f16 = np.float16
    U = np.asarray(U_z, dtype=np.float32)
    L = np.asarray(L_z, dtype=np.float32)

    # Host prep: augmented, transposed fp16 operands.
    # Contraction rows: L side [-2*L^T (32); c_hi; c_lo] vs
    #                   U side [U^T    (32);    1;    1]
    c = np.einsum("ij,ij->i", L.astype(np.float64), L.astype(np.float64))
    c_hi = c.astype(f16)
    c_lo = (c - c_hi.astype(np.float64)).astype(f16)
    lt = np.empty((K, N_L), dtype=f16)
    lt[0:NZ] = (-2.0 * L.T).astype(f16)
    lt[NZ] = c_hi
    lt[NZ + 1] = c_lo
    # Block per group so each DMA reads one contiguous [K, NCHUNK] slab.
    lt_blocked = np.ascontiguousarray(
        lt.reshape(K, GROUPS, NCHUNK).transpose(1, 0, 2)
    )

    in_maps = []
    for i in range(CORES):
        ut = np.empty((K, SHARD), dtype=f16)
        ut[0:NZ] = U[i * SHARD : (i + 1) * SHARD].T.astype(f16)
        ut[NZ] = f16(1.0)
        ut[NZ + 1] = f16(1.0)
        in_maps.append({"ut": np.ascontiguousarray(ut), "lt": lt_blocked})

    nc = _get_nc()
    kwargs = {}
    if TRACE:
        import os
        import shutil

        tdir = "/root/problem/trace_out"
        shutil.rmtree(tdir, ignore_errors=True)
        os.makedirs(tdir, exist_ok=True)
        kwargs["tmpdir"] = tdir
    res = run_bass_kernel_spmd(nc, in_maps, list(range(CORES)), trace=TRACE, **kwargs)
    LAST["exec_time_ns"] = res.exec_time_ns
    LAST["results"] = res

    # Gather: cross-partition min on host, then the cheap scalar tail.
    minval = np.empty(N_U, dtype=np.float32)
    for i in range(CORES):
        pm = res.results[i]["partmin"]  # [128, SHARD]
        minval[i * SHARD : (i + 1) * SHARD] = pm.min(axis=0)

    u_sq = np.einsum("ij,ij->i", U, U, dtype=np.float32)
    d2 = np.maximum(u_sq + minval, 0.0).astype(np.float32)
    div = np.sqrt(d2)
    dens = (-0.5 * u_sq - 0.5 * NZ * LOG_2PI).astype(np.float32)
    dd = np.exp(dens + np.log(div + EPS)).astype(np.float32)
    dd = dd - dd.min()
    dd = dd / (dd.max() + np.float32(EPS))
    return dd.astype(np.float32)


# revision 36
# speedup vs baseline: 1.0114x; 1.0114x over previous
"""DiversityDensity kernel for 8x Trainium2 NeuronCores.

Math: for each row u of U_z:
    dens(u)  = -0.5*||u||^2 - 0.5*NZ*log(2*pi)
    div(u)   = min_l ||u - l||_2  over rows l of L_z
    dd       = exp(dens + log(div + eps)); dd = (dd - min dd) / (max dd + eps)

Device computes m(u) = min_l (||l||^2 - 2 u.l) via a K=34 fp16 matmul
(32 features + split ||l||^2 hi/lo rows against a ones pair) streamed
over L in 512-row chunks, with a running elementwise min over the PSUM
tiles.  Layout: L-rows on output partitions (M), u on the free dim (N),
so the final reduce is a cheap cross-partition min done on host after
gather.

The PE in this environment is pinned at the cold 1.2 GHz HAM clock, so
K=34 matmuls use 64x64 array tiling: four M=64 matmuls (two groups x
two PSUM partition halves) co-stream in the four array quadrants,
recovering 4x column throughput.  Each PSUM tile interleaves TWO
groups' chunks in its column blocks so a full 4-tile quad fills ONE
psum tile and the 2-slot fill/drain pipeline survives.

PSUM drain is the throughput bound; it is split across two routes:
  A: DVE tensor_tensor(min) fp32 straight from PSUM into acc32
  B: ScalarE copy PSUM->SBUF fp16, then DVE min at 2x into acc16
with N_A tuned so DVE and ScalarE finish together.
d(u)^2 = ||u||^2 + m(u); sqrt/exp/normalize on host (O(N_U) work).

Sharding: U_z rows split 8 ways (512 rows/core); L_z (8 MB) replicated.
Measured: ~246 us HW exec, L2 rel err ~6e-5 (fp16 operand rounding).
"""

import numpy as np

N_U, N_L, NZ = 4096, 65536, 32
CORES = 8
SHARD = N_U // CORES  # 512
K = NZ + 2  # 34: 32 features + c_hi + c_lo rows
NCHUNK = 512  # L-rows per group (4 matmuls of M=128)
GROUPS = N_L // NCHUNK  # 128
LOG_2PI = float(np.log(2.0 * np.pi))
EPS = 1e-18

# Per-group drain route: A = DVE tensor_tensor(min) fp32 from PSUM,
# B = ScalarE copy to SBUF fp16 + DVE min at 2x.  Balances DVE vs ACT.
N_A = 33

TRACE = False
LAST = {}

_CACHE = {}


def _route_a(g: int) -> bool:
    return (g + 1) * N_A // GROUPS > g * N_A // GROUPS


def _build():
    import concourse.bass as bass  # noqa: F401
    import concourse.tile as tile
    from concourse import bacc, mybir

    f32 = mybir.dt.float32
    f16 = mybir.dt.float16
    MIN = mybir.AluOpType.min

    nc = bacc.Bacc(
        "TRN2", target_bir_lowering=False, debug=False, num_devices=CORES
    )
    ut_d = nc.declare_dram_parameter("ut", [K, SHARD], f16, isOutput=False)
    lt_d = nc.declare_dram_parameter("lt", [GROUPS, K, NCHUNK], f16, isOutput=False)
    out_d = nc.declare_dram_parameter("partmin", [128, SHARD], f32, isOutput=True)

    with tile.TileContext(nc) as tc:
        with (
            tc.tile_pool(name="const", bufs=1) as cpool,
            tc.tile_pool(name="ltp", bufs=10) as ltpool,
            tc.tile_pool(name="accp", bufs=1) as accpool,
            tc.tile_pool(name="s16p", bufs=10) as s16pool,
            tc.tile_pool(name="tailp", bufs=1) as tailpool,
            tc.tile_pool(name="psum", bufs=2, space="PSUM") as pspool,
        ):
            # rhs (U^T in fp16, plus two ones rows) at both PE row-group bases
            ut_t = cpool.tile([64 + K, SHARD], f16)
            nc.sync.dma_start(ut_t[0:K, :], ut_d[:, :])
            nc.sync.dma_start(ut_t[64 : 64 + K, :], ut_d[:, :])

            # Two fp16 accumulators alternate across route-B chunks so
            # consecutive DVE mins don't RAW-chain (hides the DVE pipe
            # drain); memsets go to GpSimd to keep DVE free.
            acc32 = accpool.tile([128, 2048], f32)
            acc16a = accpool.tile([128, 2048], f16)
            acc16b = accpool.tile([128, 2048], f16)
            nc.gpsimd.memset(acc32[:], 3.0e38)
            nc.gpsimd.memset(acc16a[:], 60000.0)
            nc.gpsimd.memset(acc16b[:], 60000.0)
            accs16 = (acc16a, acc16b)
            nb = 0

            # 64x64 array tiling: each 128-l-row chunk becomes two M=64
            # matmuls into disjoint PSUM partition halves; even/odd groups
            # sit in SBUF partition halves 0/64.  Emitting group PAIRS with
            # chunk-interleaved matmuls puts all four array tiles
            # (0,0),(0,64),(64,0),(64,64) in flight -> 4x co-streaming,
            # recovering full throughput at the cold PE clock.
            for gp in range(GROUPS // 2):
                g0, g1 = 2 * gp, 2 * gp + 1
                lt0 = ltpool.tile([64 + K, NCHUNK], f16, tag="lt")
                nc.sync.dma_start(lt0[0:K, :], lt_d[g0])
                lt1 = ltpool.tile([64 + K, NCHUNK], f16, tag="lt")
                nc.sync.dma_start(lt1[64 : 64 + K, :], lt_d[g1])

                # Each psum tile holds TWO chunks of EACH group (column
                # blocks: [g0.c, g0.c+1, g1.c, g1.c+1]) so a full 4-tile
                # quad streams into ONE psum tile and the 2-slot pipeline
                # (PE fills one tile while the other drains) is preserved.
                for half in range(2):
                    ps = pspool.tile([128, 2048], f32, tag="ps")
                    for cc in range(2):
                        c = 2 * half + cc
                        for bi, (base, lt_t) in enumerate(((0, lt0), (64, lt1))):
                            for h in range(2):
                                nc.tensor.matmul(
                                    ps[
                                        64 * h : 64 * h + 64,
                                        (2 * bi + cc) * 512 : (2 * bi + cc + 1) * 512,
                                    ],
                                    lhsT=lt_t[
                                        base : base + K,
                                        c * 128 + 64 * h : c * 128 + 64 * h + 64,
                                    ],
                                    rhs=ut_t[base : base + K, :],
                                    start=True,
                                    stop=True,
                                )
                    if _route_a(2 * gp + half):
                        nc.vector.tensor_tensor(acc32[:], acc32[:], ps[:], MIN)
                    else:
                        s16 = s16pool.tile([128, 2048], f16, tag="s16")
                        nc.scalar.copy(s16[:], ps[:])
                        acc = accs16[nb]
                        nb ^= 1
                        nc.vector.tensor_tensor(acc[:], acc[:], s16[:], MIN)

            # Tail: fold the 4 sub-chunk columns together, merge routes.
            acc16 = tailpool.tile([128, 2048], f16)
            nc.vector.tensor_tensor(acc16[:], acc16a[:], acc16b[:], MIN)
            t16a = tailpool.tile([128, 1024], f16)
            nc.vector.tensor_tensor(t16a[:], acc16[:, 0:1024], acc16[:, 1024:2048], MIN)
            t16b = tailpool.tile([128, 512], f16)
            nc.vector.tensor_tensor(t16b[:], t16a[:, 0:512], t16a[:, 512:1024], MIN)
            t16c = tailpool.tile([128, 512], f32)
            nc.vector.tensor_copy(t16c[:], t16b[:])

            t32a = tailpool.tile([128, 1024], f32)
            nc.vector.tensor_tensor(t32a[:], acc32[:, 0:1024], acc32[:, 1024:2048], MIN)
            t32b = tailpool.tile([128, 512], f32)
            nc.vector.tensor_tensor(t32b[:], t32a[:, 0:512], t32a[:, 512:1024], MIN)

            res = tailpool.tile([128, 512], f32)
            nc.vector.tensor_tensor(res[:], t32b[:], t16c[:], MIN)
            nc.sync.dma_start(out_d[:, :], res[:])

    nc.compile()
    return nc


def _get_nc():
    if "nc" not in _CACHE:
        _CACHE["nc"] = _build()
    return _CACHE["nc"]


def kernel(pred: np.ndarray, U_z: np.ndarray, L_z: np.ndarray) -> np.ndarray:
    from concourse.bass_utils import run_bass_kernel_spmd

    f16 = np.float16
    U = np.asarray(U_z, dtype=np.float32)
    L = np.asarray(L_z, dtype=np.float32)

    # Host prep: augmented, transposed fp16 operands.
    # Contraction rows: L side [-2*L^T (32); c_hi; c_lo] vs
    #                   U side [U^T    (32);    1;    1]
    c = np.einsum("ij,ij->i", L.astype(np.float64), L.astype(np.float64))
    c_hi = c.astype(f16)
    c_lo = (c - c_hi.astype(np.float64)).astype(f16)
    lt = np.empty((K, N_L), dtype=f16)
    lt[0:NZ] = (-2.0 * L.T).astype(f16)
    lt[NZ] = c_hi
    lt[NZ + 1] = c_lo
    # Block per group so each DMA reads one contiguous [K, NCHUNK] slab.
    lt_blocked = np.ascontiguousarray(
        lt.reshape(K, GROUPS, NCHUNK).transpose(1, 0, 2)
    )

    in_maps = []
    for i in range(CORES):
        ut = np.empty((K, SHARD), dtype=f16)
        ut[0:NZ] = U[i * SHARD : (i + 1) * SHARD].T.astype(f16)
        ut[NZ] = f16(1.0)
        ut[NZ + 1] = f16(1.0)
        in_maps.append({"ut": np.ascontiguousarray(ut), "lt": lt_blocked})

    nc = _get_nc()
    kwargs = {}
    if TRACE:
        import os
        import shutil

        tdir = "/root/problem/trace_out"
        shutil.rmtree(tdir, ignore_errors=True)
        os.makedirs(tdir, exist_ok=True)
        kwargs["tmpdir"] = tdir
    res = run_bass_kernel_spmd(nc, in_maps, list(range(CORES)), trace=TRACE, **kwargs)
    LAST["exec_time_ns"] = res.exec_time_ns
    LAST["results"] = res

    # Gather: cross-partition min on host, then the cheap scalar tail.
    minval = np.empty(N_U, dtype=np.float32)
    for i in range(CORES):
        pm = res.results[i]["partmin"]  # [128, SHARD]
        minval[i * SHARD : (i + 1) * SHARD] = pm.min(axis=0)

    u_sq = np.einsum("ij,ij->i", U, U, dtype=np.float32)
    d2 = np.maximum(u_sq + minval, 0.0).astype(np.float32)
    div = np.sqrt(d2)
    dens = (-0.5 * u_sq - 0.5 * NZ * LOG_2PI).astype(np.float32)
    dd = np.exp(dens + np.log(div + EPS)).astype(np.float32)
    dd = dd - dd.min()
    dd = dd / (dd.max() + np.float32(EPS))
    return dd.astype(np.float32)


# revision 37
# speedup vs baseline: 1.0135x; 1.0021x over previous
"""DiversityDensity kernel for 8x Trainium2 NeuronCores.

Math: for each row u of U_z:
    dens(u)  = -0.5*||u||^2 - 0.5*NZ*log(2*pi)
    div(u)   = min_l ||u - l||_2  over rows l of L_z
    dd       = exp(dens + log(div + eps)); dd = (dd - min dd) / (max dd + eps)

Device computes m(u) = min_l (||l||^2 - 2 u.l) via a K=34 fp16 matmul
(32 features + split ||l||^2 hi/lo rows against a ones pair) streamed
over L in 512-row chunks, with a running elementwise min over the PSUM
tiles.  Layout: L-rows on output partitions (M), u on the free dim (N),
so the final reduce is a cheap cross-partition min done on host after
gather.

The PE in this environment is pinned at the cold 1.2 GHz HAM clock, so
K=34 matmuls use 64x64 array tiling: four M=64 matmuls (two groups x
two PSUM partition halves) co-stream in the four array quadrants,
recovering 4x column throughput.  Each PSUM tile interleaves TWO
groups' chunks in its column blocks so a full 4-tile quad fills ONE
psum tile and the 2-slot fill/drain pipeline survives.

PSUM drain is the throughput bound; it is split across two routes:
  A: DVE tensor_tensor(min) fp32 straight from PSUM into acc32
  B: ScalarE copy PSUM->SBUF fp16, then DVE min at 2x into acc16
with N_A tuned so DVE and ScalarE finish together.
d(u)^2 = ||u||^2 + m(u); sqrt/exp/normalize on host (O(N_U) work).

Sharding: U_z rows split 8 ways (512 rows/core); L_z (8 MB) replicated.
Measured: ~246 us HW exec, L2 rel err ~6e-5 (fp16 operand rounding).
"""

import numpy as np

N_U, N_L, NZ = 4096, 65536, 32
CORES = 8
SHARD = N_U // CORES  # 512
K = NZ + 2  # 34: 32 features + c_hi + c_lo rows
NCHUNK = 512  # L-rows per group (4 matmuls of M=128)
GROUPS = N_L // NCHUNK  # 128
LOG_2PI = float(np.log(2.0 * np.pi))
EPS = 1e-18

# Per-group drain route: A = DVE tensor_tensor(min) fp32 from PSUM,
# B = ScalarE copy to SBUF fp16 + DVE min at 2x.  Balances DVE vs ACT.
N_A = 33

TRACE = False
LAST = {}

_CACHE = {}


def _route_a(g: int) -> bool:
    return (g + 1) * N_A // GROUPS > g * N_A // GROUPS


def _build():
    import concourse.bass as bass  # noqa: F401
    import concourse.tile as tile
    from concourse import bacc, mybir

    f32 = mybir.dt.float32
    f16 = mybir.dt.float16
    MIN = mybir.AluOpType.min

    nc = bacc.Bacc(
        "TRN2", target_bir_lowering=False, debug=False, num_devices=CORES
    )
    ut_d = nc.declare_dram_parameter("ut", [K, SHARD], f16, isOutput=False)
    lt_d = nc.declare_dram_parameter("lt", [GROUPS, K, NCHUNK], f16, isOutput=False)
    out_d = nc.declare_dram_parameter("partmin", [128, SHARD], f32, isOutput=True)

    with tile.TileContext(nc) as tc:
        with (
            tc.tile_pool(name="const", bufs=1) as cpool,
            tc.tile_pool(name="ltp", bufs=10) as ltpool,
            tc.tile_pool(name="accp", bufs=1) as accpool,
            tc.tile_pool(name="s16p", bufs=10) as s16pool,
            tc.tile_pool(name="tailp", bufs=1) as tailpool,
            tc.tile_pool(name="psum", bufs=2, space="PSUM") as pspool,
        ):
            # rhs (U^T in fp16, plus two ones rows) at both PE row-group bases
            ut_t = cpool.tile([64 + K, SHARD], f16)
            nc.sync.dma_start(ut_t[0:K, :], ut_d[:, :])
            nc.sync.dma_start(ut_t[64 : 64 + K, :], ut_d[:, :])

            # Two fp16 accumulators alternate across route-B chunks so
            # consecutive DVE mins don't RAW-chain (hides the DVE pipe
            # drain); memsets go to GpSimd to keep DVE free.
            acc32 = accpool.tile([128, 2048], f32)
            acc16a = accpool.tile([128, 2048], f16)
            acc16b = accpool.tile([128, 2048], f16)
            nc.gpsimd.memset(acc32[:], 3.0e38)
            nc.gpsimd.memset(acc16a[:], 60000.0)
            nc.gpsimd.memset(acc16b[:], 60000.0)
            accs16 = (acc16a, acc16b)
            nb = 0
            pend = [None]  # deferred route-B min: (acc, s16)

            # 64x64 array tiling: each 128-l-row chunk becomes two M=64
            # matmuls into disjoint PSUM partition halves; even/odd groups
            # sit in SBUF partition halves 0/64.  Emitting group PAIRS with
            # chunk-interleaved matmuls puts all four array tiles
            # (0,0),(0,64),(64,0),(64,64) in flight -> 4x co-streaming,
            # recovering full throughput at the cold PE clock.
            for gp in range(GROUPS // 2):
                g0, g1 = 2 * gp, 2 * gp + 1
                lt0 = ltpool.tile([64 + K, NCHUNK], f16, tag="lt")
                nc.sync.dma_start(lt0[0:K, :], lt_d[g0])
                lt1 = ltpool.tile([64 + K, NCHUNK], f16, tag="lt")
                nc.sync.dma_start(lt1[64 : 64 + K, :], lt_d[g1])

                # Each psum tile holds TWO chunks of EACH group (column
                # blocks: [g0.c, g0.c+1, g1.c, g1.c+1]) so a full 4-tile
                # quad streams into ONE psum tile and the 2-slot pipeline
                # (PE fills one tile while the other drains) is preserved.
                for half in range(2):
                    ps = pspool.tile([128, 2048], f32, tag="ps")
                    for cc in range(2):
                        c = 2 * half + cc
                        for bi, (base, lt_t) in enumerate(((0, lt0), (64, lt1))):
                            for h in range(2):
                                nc.tensor.matmul(
                                    ps[
                                        64 * h : 64 * h + 64,
                                        (2 * bi + cc) * 512 : (2 * bi + cc + 1) * 512,
                                    ],
                                    lhsT=lt_t[
                                        base : base + K,
                                        c * 128 + 64 * h : c * 128 + 64 * h + 64,
                                    ],
                                    rhs=ut_t[base : base + K, :],
                                    start=True,
                                    stop=True,
                                )
                    if _route_a(2 * gp + half):
                        nc.vector.tensor_tensor(acc32[:], acc32[:], ps[:], MIN)
                        if pend[0] is not None:
                            acc, s16 = pend[0]
                            pend[0] = None
                            nc.vector.tensor_tensor(acc[:], acc[:], s16[:], MIN)
                    else:
                        s16 = s16pool.tile([128, 2048], f16, tag="s16")
                        nc.scalar.copy(s16[:], ps[:])
                        # Software-pipeline: DVE mins the PREVIOUS tile's
                        # copy now (it finished a full tile ago), deferring
                        # this one — kills the in-order DVE queue's
                        # head-of-line wait on the ScalarE copy.
                        if pend[0] is not None:
                            acc, s16p = pend[0]
                            nc.vector.tensor_tensor(acc[:], acc[:], s16p[:], MIN)
                        acc = accs16[nb]
                        nb ^= 1
                        pend[0] = (acc, s16)

            # Tail: flush the deferred min, fold columns, merge routes.
            if pend[0] is not None:
                acc, s16p = pend[0]
                nc.vector.tensor_tensor(acc[:], acc[:], s16p[:], MIN)
            acc16 = tailpool.tile([128, 2048], f16)
            nc.vector.tensor_tensor(acc16[:], acc16a[:], acc16b[:], MIN)
            t16a = tailpool.tile([128, 1024], f16)
            nc.vector.tensor_tensor(t16a[:], acc16[:, 0:1024], acc16[:, 1024:2048], MIN)
            t16b = tailpool.tile([128, 512], f16)
            nc.vector.tensor_tensor(t16b[:], t16a[:, 0:512], t16a[:, 512:1024], MIN)
            t16c = tailpool.tile([128, 512], f32)
            nc.vector.tensor_copy(t16c[:], t16b[:])

            t32a = tailpool.tile([128, 1024], f32)
            nc.vector.tensor_tensor(t32a[:], acc32[:, 0:1024], acc32[:, 1024:2048], MIN)
            t32b = tailpool.tile([128, 512], f32)
            nc.vector.tensor_tensor(t32b[:], t32a[:, 0:512], t32a[:, 512:1024], MIN)

            res = tailpool.tile([128, 512], f32)
            nc.vector.tensor_tensor(res[:], t32b[:], t16c[:], MIN)
            nc.sync.dma_start(out_d[:, :], res[:])

    nc.compile()
    return nc


def _get_nc():
    if "nc" not in _CACHE:
        _CACHE["nc"] = _build()
    return _CACHE["nc"]


def kernel(pred: np.ndarray, U_z: np.ndarray, L_z: np.ndarray) -> np.ndarray:
    from concourse.bass_utils import run_bass_kernel_spmd

    f16 = np.float16
    U = np.asarray(U_z, dtype=np.float32)
    L = np.asarray(L_z, dtype=np.float32)

    # Host prep: augmented, transposed fp16 operands.
    # Contraction rows: L side [-2*L^T (32); c_hi; c_lo] vs
    #                   U side [U^T    (32);    1;    1]
    c = np.einsum("ij,ij->i", L.astype(np.float64), L.astype(np.float64))
    c_hi = c.astype(f16)
    c_lo = (c - c_hi.astype(np.float64)).astype(f16)
    lt = np.empty((K, N_L), dtype=f16)
    lt[0:NZ] = (-2.0 * L.T).astype(f16)
    lt[NZ] = c_hi
    lt[NZ + 1] = c_lo
    # Block per group so each DMA reads one contiguous [K, NCHUNK] slab.
    lt_blocked = np.ascontiguousarray(
        lt.reshape(K, GROUPS, NCHUNK).transpose(1, 0, 2)
    )

    in_maps = []
    for i in range(CORES):
        ut = np.empty((K, SHARD), dtype=f16)
        ut[0:NZ] = U[i * SHARD : (i + 1) * SHARD].T.astype(f16)
        ut[NZ] = f16(1.0)
        ut[NZ + 1] = f16(1.0)
        in_maps.append({"ut": np.ascontiguousarray(ut), "lt": lt_blocked})

    nc = _get_nc()
    kwargs = {}
    if TRACE:
        import os
        import shutil

        tdir = "/root/problem/trace_out"
        shutil.rmtree(tdir, ignore_errors=True)
        os.makedirs(tdir, exist_ok=True)
        kwargs["tmpdir"] = tdir
    res = run_bass_kernel_spmd(nc, in_maps, list(range(CORES)), trace=TRACE, **kwargs)
    LAST["exec_time_ns"] = res.exec_time_ns
    LAST["results"] = res

    # Gather: cross-partition min on host, then the cheap scalar tail.
    minval = np.empty(N_U, dtype=np.float32)
    for i in range(CORES):
        pm = res.results[i]["partmin"]  # [128, SHARD]
        minval[i * SHARD : (i + 1) * SHARD] = pm.min(axis=0)

    u_sq = np.einsum("ij,ij->i", U, U, dtype=np.float32)
    d2 = np.maximum(u_sq + minval, 0.0).astype(np.float32)
    div = np.sqrt(d2)
    dens = (-0.5 * u_sq - 0.5 * NZ * LOG_2PI).astype(np.float32)
    dd = np.exp(dens + np.log(div + EPS)).astype(np.float32)
    dd = dd - dd.min()
    dd = dd / (dd.max() + np.float32(EPS))
    return dd.astype(np.float32)
